# revision 81
# baseline (speedup 1.0000x reference)
"""Trainium2 Bass kernel for nn_BloqueAttn: causal RoPE attention, 16 heads,
head-sharded (tensor-parallel) across 8 NeuronCores, o_proj row-sharded with
host-side all-reduce of the partials.

v2: bf16 datapath, query-on-partition PV (65-wide moving operand), PE
perm-matmul RoPE swap, mask-by-multiply on DVE, per-partition softmax
normalization, batched DMAs with host-side pre-layout.

Self-contained: hardcodes shapes B=1, L=4096, D=1024, H=16, DH=64, 8 cores.
"""
import os

os.environ.setdefault("BASS_NEVER_TRACE", "1")

import numpy as np
import ml_dtypes

import concourse.bass as bass
import concourse.bacc as bacc
import concourse.mybir as mybir
import concourse.tile as tile
from concourse.bass_utils import run_bass_kernel_spmd

F32 = mybir.dt.float32
BF16 = mybir.dt.bfloat16
I16 = mybir.dt.int16

B, L, D = 1, 4096, 1024
H, DH = 16, 64
BASE = 10000.0
N_CORES = 8
HPC = H // N_CORES          # heads per core = 2
DH2 = HPC * DH              # packed head dim = 128
SCALE = DH ** -0.5          # 0.125

# Schraudolph-style exp in bf16 bits: bf16(e^(x*SCALE)) ~= bits of
# int16(A*x + B) with A = SCALE * 2^7 / ln2, B = 127*2^7 - 7.41 (minimax).
SCH_A = SCALE * 128.0 / np.log(2.0)
SCH_B = 16256.0 - 7.41


# ---------------------------------------------------------------- host helpers

def _rope_tables(L_, dh):
    inv_freq = 1.0 / (BASE ** (np.arange(0, dh, 2, dtype=np.float32) / dh))
    freqs = np.outer(np.arange(L_, dtype=np.float32), inv_freq)  # [L, 32]
    return np.cos(freqs).astype(np.float32), np.sin(freqs).astype(np.float32)


def _host_consts(L_):
    cos, sin = _rope_tables(L_, DH)          # [L, 32]
    cosT, sinT = cos.T.copy(), sin.T.copy()  # [32, L]
    cos_stack = np.concatenate([cosT, cosT, cosT, cosT], 0)          # [128, L]
    sin_signed = np.concatenate([-sinT, sinT, -sinT, sinT], 0)       # [128, L]

    # 0/1 causal keep-mask within a 128x128 diagonal block:
    # key j visible to query c iff j <= c.
    j = np.arange(128)[:, None]
    c = np.arange(128)[None, :]
    tril01 = (j <= c).astype(np.float32)                             # [128,128]

    ident = np.eye(128, dtype=np.float32)
    # 32-row block swap permutation: out[i] = in[sigma(i)],
    # sigma = [32..63, 0..31, 96..127, 64..95]
    sigma = np.concatenate([np.arange(32, 64), np.arange(0, 32),
                            np.arange(96, 128), np.arange(64, 96)])
    pmat = np.zeros((128, 128), np.float32)
    pmat[sigma, np.arange(128)] = 1.0        # out = pmat.T @ in
    return {
        "cos_st": cos_stack.astype(ml_dtypes.bfloat16),
        "sin_st": sin_signed.astype(ml_dtypes.bfloat16),
        "tril01": tril01.astype(ml_dtypes.bfloat16),
        "ident_b": ident.astype(ml_dtypes.bfloat16),
        "perm_b": pmat.astype(ml_dtypes.bfloat16),
    }


def _chunk_major(wT):
    """[D, 128] -> [128, D] with 128-row chunks laid side by side."""
    ndc = wT.shape[0] // 128
    return np.ascontiguousarray(
        wT.reshape(ndc, 128, 128).transpose(1, 0, 2).reshape(128, ndc * 128))


def _core_weights(core, Wq, Wk, Wv, Wo):
    """Per-core weight slices, bf16, chunk-major; RoPE even/odd permutation
    applied to Wq/Wk rows."""
    perm = np.concatenate([np.arange(0, DH, 2), np.arange(1, DH, 2)])  # [64]
    rows_p, rows = [], []
    for hh in (HPC * core, HPC * core + 1):
        rows_p.append(DH * hh + perm)
        rows.append(DH * hh + np.arange(DH))
    rows_p = np.concatenate(rows_p)
    rows = np.concatenate(rows)
    wq = _chunk_major(Wq[rows_p, :].T).astype(ml_dtypes.bfloat16)  # [128, 1024]
    wk = _chunk_major(Wk[rows_p, :].T).astype(ml_dtypes.bfloat16)
    wv = _chunk_major(Wv[rows, :].T).astype(ml_dtypes.bfloat16)
    woC = np.ascontiguousarray(
        Wo[:, DH2 * core: DH2 * (core + 1)].T).astype(ml_dtypes.bfloat16)
    return wq, wk, wv, woC


def _layout_x(x, L_):
    """x [B,L,D] -> [128, 8*4096] bf16, 512-col subtile-major:
    xr[p, s*4096 + ch*512 + c] = x[s*512+c, ch*128+p]."""
    xT = np.ascontiguousarray(x.reshape(L_, D).T)        # [D, L]
    ns = L_ // 512
    xr = xT.reshape(8, 128, ns, 512).transpose(1, 2, 0, 3)
    return np.ascontiguousarray(xr.reshape(128, ns * 4096)).astype(
        ml_dtypes.bfloat16)


# ---------------------------------------------------------------- device emit

def emit(nc, tc, aps, L_):
    NSB = L_ // 512           # 512-col subtiles (8) == query blocks
    NQB = L_ // 512
    NKB = L_ // 128           # key blocks (32)
    ND = D // 128             # D chunks (8)

    xt = aps["xt"]
    partial = aps["partial"]
    ACT_EXP = mybir.ActivationFunctionType.Exp

    with tc.tile_pool(name="persist", bufs=1) as pp, \
         tc.tile_pool(name="psB", bufs=1, space="PSUM") as psB, \
         tc.tile_pool(name="psS", bufs=1, space="PSUM") as psS, \
         tc.tile_pool(name="sbC", bufs=1) as sbC, \
         tc.tile_pool(name="sbB", bufs=1) as sbB, \
         tc.tile_pool(name="sbA", bufs=1) as sbA:
        wq_sb = pp.tile([128, D], BF16)
        wk_sb = pp.tile([128, D], BF16)
        wv_sb = pp.tile([128, D], BF16)
        wo_sb = pp.tile([128, D], BF16)
        cos_sb = pp.tile([128, L_], BF16)
        sin_sb = pp.tile([128, L_], BF16)
        tril_sb = pp.tile([128, 128], BF16)
        idb_sb = pp.tile([128, 128], BF16)
        perm_sb = pp.tile([128, 128], BF16)
        qT = pp.tile([128, L_], BF16)
        kT = pp.tile([128, L_], BF16)
        v_sb = pp.tile([128, NKB * 130], BF16)
        # PE pstate warmup: the cost model ramps the PE clock over 3us from
        # the first matmul; dummy matmuls during the initial DMA wait start
        # the ramp early so real work runs at mid/full speed sooner.
        wup = pp.tile([128, 256], BF16)
        nc.gpsimd.memset(wup[:], 0.0)
        nc.sync.dma_start(wq_sb[:], aps["wq"][:])
        nc.gpsimd.memset(v_sb[:], 1.0)   # ones columns for the sum trick
        for _ in range(3):
            wps = psS.tile([128, 512], F32, tag="scr", bufs=2)
            nc.tensor.matmul(wps[:, 0:256], wup[:, 0:128], wup[:],
                             start=True, stop=True)

        def phase_a(s):
            """Projections + RoPE + V transpose for L-subtile s (512 cols)."""
            sl = bass.ds(512 * s, 512)
            xt_t = sbA.tile([128, 4096], BF16, tag="xt", bufs=2)
            if s == 0:
                for ch in range(ND):
                    nc.sync.dma_start(xt_t[:, bass.ts(ch, 512)],
                                      xt[:, bass.ds(ch * 512, 512)])
                nc.sync.dma_start(wk_sb[:], aps["wk"][:])
                nc.sync.dma_start(wv_sb[:], aps["wv"][:])
                # only the first 512-col slice of cos/sin is needed for s=0;
                # the rest streams in behind xt(1) to unblock it
                nc.sync.dma_start(cos_sb[:, 0:512], aps["cos_st"][:, 0:512])
                nc.sync.dma_start(sin_sb[:, 0:512], aps["sin_st"][:, 0:512])
                nc.sync.dma_start(perm_sb[:], aps["perm_b"][:])
                nc.sync.dma_start(idb_sb[:], aps["ident_b"][:])
                nc.sync.dma_start(tril_sb[:], aps["tril01"][:])
            else:
                nc.sync.dma_start(xt_t[:], xt[:, bass.ts(s, 4096)])
                if s == 1:
                    nc.sync.dma_start(cos_sb[:, 512:L_],
                                      aps["cos_st"][:, 512:L_])
                    nc.sync.dma_start(sin_sb[:, 512:L_],
                                      aps["sin_st"][:, 512:L_])
                    nc.sync.dma_start(wo_sb[:], aps["wo"][:])
            raws = {}
            # q/k projections first so their RoPE (DVE) overlaps the
            # V projection + transposes (PE) and attention(s) starts clean.
            for name, wsb in (("q", wq_sb), ("k", wk_sb)):
                ps = psS.tile([128, 512], F32, tag="scr", bufs=2)
                for ch in range(ND):
                    nc.tensor.matmul(ps[:], wsb[:, bass.ts(ch, 128)],
                                     xt_t[:, bass.ts(ch, 512)],
                                     start=ch == 0, stop=ch == ND - 1)
                raw = sbA.tile([128, 512], BF16, tag=f"raw{name}", bufs=2)
                if s <= 3:
                    nc.scalar.copy(raw[:], ps[:])   # ACT is starved early
                else:
                    nc.vector.tensor_copy(raw[:], ps[:])
                raws[name] = raw
            # RoPE: rot = raw*cos + perm(raw)*sin_signed
            for name, dst in (("q", qT), ("k", kT)):
                raw = raws[name]
                aux = psS.tile([128, 512], F32, tag="scr", bufs=2)
                nc.tensor.matmul(aux[:], perm_sb[:], raw[:],
                                 start=True, stop=True)
                swp = sbA.tile([128, 512], BF16, tag="swp", bufs=2)
                nc.vector.tensor_mul(swp[:], aux[:], sin_sb[:, sl])
                nc.vector.tensor_mul(dst[:, sl], raw[:], cos_sb[:, sl])
                nc.vector.tensor_add(dst[:, sl], dst[:, sl], swp[:])
            psv = psS.tile([128, 512], F32, tag="scr", bufs=2)
            for ch in range(ND):
                nc.tensor.matmul(psv[:], wv_sb[:, bass.ts(ch, 128)],
                                 xt_t[:, bass.ts(ch, 512)],
                                 start=ch == 0, stop=ch == ND - 1)
            vt = sbA.tile([128, 512], BF16, tag="rawv", bufs=2)
            nc.vector.tensor_copy(vt[:], psv[:])
            # V transpose into [key, dh] layout with ones columns:
            # v_sb[:, 130*kb + {0..63, 65..128}], kb = 4*s + j
            auxv_t = psS.tile([128, 512], F32, tag="scr", bufs=2)
            auxv = auxv_t[:].bitcast(BF16)[:, 0:512]
            for j in range(4):
                nc.tensor.transpose(auxv[:, bass.ts(j, 128)],
                                    vt[:, bass.ts(j, 128)], idb_sb[:])
            src = auxv.rearrange("p (j h c) -> p j h c", j=4, h=2)
            vdst = v_sb[:, bass.ds(130 * 4 * s, 130 * 4)].rearrange(
                "p (j h c) -> p j h c", j=4, c=65)[:, :, :, 0:64]
            nc.vector.tensor_copy(vdst, src)

        state = {}  # qb -> (O_sb, invs) for deferred norm/o_proj

        def norm_oproj(qb, use_act=False):
            O_t, invs = state.pop(qb)
            # use_act: ACT is free of exp work here; alternate ACT/DVE so
            # neither engine serializes the whole copy chain
            def cpy(dst, src, n=[0]):
                n[0] ^= 1
                if use_act and n[0]:
                    nc.scalar.copy(dst, src)
                else:
                    nc.vector.tensor_copy(dst, src)
            for qs in range(4):
                lc = 4 * qb + qs
                for h in range(2):
                    i = 2 * qs + h
                    nc.vector.tensor_scalar_mul(
                        O_t[:, bass.ds(64 * i, 64)],
                        O_t[:, bass.ds(64 * i, 64)],
                        invs[:, i:i + 1])
                trp_t = psS.tile([128, 512], F32, tag="scr", bufs=2)
                trpb = trp_t[:].bitcast(BF16)[:, 0:128]
                nc.tensor.transpose(trpb, O_t[:, bass.ts(qs, 128)],
                                    idb_sb[:])
                ot_t = sbC.tile([128, 128], BF16, tag="ot", bufs=2)
                if use_act:
                    nc.scalar.copy(ot_t[:], trpb)   # ACT idle at the tail
                else:
                    nc.vector.tensor_copy(ot_t[:], trpb)
                ob = sbC.tile([128, 1024], BF16, tag="ob", bufs=4)
                for n in range(2):
                    op = psS.tile([128, 512], F32, tag="scr", bufs=2)
                    nc.tensor.matmul(op[:], ot_t[:],
                                     wo_sb[:, bass.ts(n, 512)],
                                     start=True, stop=True)
                    cpy(ob[:, bass.ts(n, 512)], op[:])
                # tail DMAs ride the idle HWDGE path; mid-stream ones stay on
                # SWDGE to keep HWDGE free for input streaming
                if use_act:
                    nc.sync.dma_start(partial[bass.ts(lc, 128), :], ob[:])
                else:
                    nc.gpsimd.dma_start(partial[bass.ts(lc, 128), :], ob[:])

        # PSUM accumulate-group state is per bank: region 7 would cross
        # the 2048B bank boundary at col 455, so it lives at col 512.
        PVC = [65 * i for i in range(7)] + [512]

        def attention(qb):
            qsl0 = 512 * qb
            pvacc = psB.tile([128, 577], F32, tag="pv", bufs=1)
            # no zeroing matmuls: the first PV into each bank (kb=0, i=0 for
            # bank 0 / i=7 for bank 1) carries start=True, clearing the
            # bank's has_written; later regions then write in overwrite mode
            nkb = 4 * qb + 4
            for kb in range(nkb):
                r = kb - 4 * qb
                c0 = 128 * r if r > 0 else 0
                W = 512 - c0
                ksl = bass.ts(kb, 128)
                qsl = bass.ds(qsl0 + c0, W)
                s01 = psB.tile([128, 1024], F32, tag="sc", bufs=2)
                nc.tensor.matmul(s01[:, c0:512], kT[0:64, ksl],
                                 qT[0:64, qsl], start=True, stop=True)
                nc.tensor.matmul(s01[:, 512 + c0:1024], kT[64:128, ksl],
                                 qT[64:128, qsl], start=True, stop=True)
                p01 = sbB.tile([128, 1024], BF16, tag="p01", bufs=8)
                if qb >= 6 and r < 0 and kb % 4 == 3:
                    # offload exp to DVE via bf16 Schraudolph bit trick
                    nc.vector.tensor_scalar(
                        p01[:].bitcast(I16), s01[:], SCH_A, SCH_B,
                        mybir.AluOpType.mult, mybir.AluOpType.add)
                else:
                    sin_ = s01[:].rearrange(
                        "p (h c) -> p h c", h=2)[:, :, c0:512]
                    pout = p01[:].rearrange(
                        "p (h c) -> p h c", h=2)[:, :, c0:512]
                    nc.scalar.activation(pout, sin_, ACT_EXP, scale=SCALE)
                if r >= 0:
                    for h in range(2):
                        msl = bass.ds(512 * h + c0, 128)
                        nc.vector.tensor_mul(p01[:, msl], p01[:, msl],
                                             tril_sb[:])
                for qs in range(max(0, r), 4):
                    for h in range(2):
                        i = 2 * qs + h
                        nc.tensor.matmul(
                            pvacc[:, bass.ds(PVC[i], 65)],
                            p01[:, bass.ds(512 * h + 128 * qs, 128)],
                            v_sb[:, bass.ds(130 * kb + 65 * h, 65)],
                            start=(kb == 0 and i in (0, 7)),
                            stop=kb == 4 * qb + qs,
                            skip_group_check=True)
            # free pvacc quickly: reciprocal of sums + copy out
            invs = sbB.tile([128, 8], F32, tag="invs", bufs=3)
            sums7 = pvacc[:, 0:455].rearrange(
                "p (i c) -> p i c", c=65)[:, :, 64]
            nc.vector.reciprocal(invs[:, 0:7], sums7)
            nc.vector.reciprocal(invs[:, 7:8], pvacc[:, 576:577])
            O_t = sbB.tile([128, 512], BF16, tag="osb", bufs=3)
            psrc7 = pvacc[:, 0:455].rearrange(
                "p (i c) -> p i c", c=65)[:, :, 0:64]
            nc.vector.tensor_copy(
                O_t[:, 0:448].rearrange("p (i c) -> p i c", c=64), psrc7)
            nc.vector.tensor_copy(O_t[:, 448:512], pvacc[:, 512:576])
            state[qb] = (O_t, invs)

        # Interleave: attention row qb needs keys 0..512*(qb+1) = subtiles
        # 0..qb, so row s can run right after phase_a(s).
        for s in range(NSB):
            phase_a(s)
            attention(s)
            if s >= 1:
                norm_oproj(s - 1)
        norm_oproj(NQB - 1, use_act=True)


def build(L_=L, debug=False):
    nc = bacc.Bacc("TRN2", target_bir_lowering=False, debug=debug,
                   enable_asserts=False)
    aps = {}
    NSB = L_ // 512
    aps["xt"] = nc.dram_tensor("xt", [128, NSB * 4096], BF16,
                               kind="ExternalInput").ap()
    for w in ("wq", "wk", "wv", "wo"):
        aps[w] = nc.dram_tensor(w, [128, D], BF16, kind="ExternalInput").ap()
    aps["cos_st"] = nc.dram_tensor("cos_st", [128, L_], BF16,
                                   kind="ExternalInput").ap()
    aps["sin_st"] = nc.dram_tensor("sin_st", [128, L_], BF16,
                                   kind="ExternalInput").ap()
    aps["tril01"] = nc.dram_tensor("tril01", [128, 128], BF16,
                                   kind="ExternalInput").ap()
    aps["ident_b"] = nc.dram_tensor("ident_b", [128, 128], BF16,
                                    kind="ExternalInput").ap()
    aps["perm_b"] = nc.dram_tensor("perm_b", [128, 128], BF16,
                                   kind="ExternalInput").ap()
    aps["partial"] = nc.dram_tensor("partial", [L_, D], BF16,
                                    kind="ExternalOutput").ap()

    with tile.TileContext(nc) as tc:
        emit(nc, tc, aps, L_)
    nc.compile()
    return nc, aps


def make_in_maps(x, Wq, Wk, Wv, Wo, L_=L):
    xr = _layout_x(x, L_)
    consts = _host_consts(L_)
    in_maps = []
    for c in range(N_CORES):
        wq, wk, wv, woC = _core_weights(c, Wq, Wk, Wv, Wo)
        m = {"xt": xr, "wq": wq, "wk": wk, "wv": wv, "wo": woC}
        m.update(consts)
        in_maps.append(m)
    return in_maps


_CACHE = {}


def _run(inputs, trace=False, **kw):
    if trace:
        os.environ.pop("BASS_NEVER_TRACE", None)
    x = np.asarray(inputs["x"], np.float32)
    Wq = np.asarray(inputs["Wq"], np.float32)
    Wk = np.asarray(inputs["Wk"], np.float32)
    Wv = np.asarray(inputs["Wv"], np.float32)
    Wo = np.asarray(inputs["Wo"], np.float32)
    if "nc" not in _CACHE:
        _CACHE["nc"] = build()[0]
    nc = _CACHE["nc"]
    in_maps = make_in_maps(x, Wq, Wk, Wv, Wo)
    res = run_bass_kernel_spmd(nc, in_maps, core_ids=list(range(N_CORES)),
                               trace=trace, **kw)
    acc = np.zeros((L, D), np.float64)
    for r in res.results:
        acc += r["partial"].astype(np.float64)
    out = acc.astype(np.float32).reshape(B, L, D)
    return out, res


def kernel(**inputs):
    out, _ = _run(inputs)
    return out


# revision 82
# speedup vs baseline: 1.0040x; 1.0040x over previous
"""Trainium2 Bass kernel for nn_BloqueAttn: causal RoPE attention, 16 heads,
head-sharded (tensor-parallel) across 8 NeuronCores, o_proj row-sharded with
host-side all-reduce of the partials.

v2: bf16 datapath, query-on-partition PV (65-wide moving operand), PE
perm-matmul RoPE swap, mask-by-multiply on DVE, per-partition softmax
normalization, batched DMAs with host-side pre-layout.

Self-contained: hardcodes shapes B=1, L=4096, D=1024, H=16, DH=64, 8 cores.
"""
import os

os.environ.setdefault("BASS_NEVER_TRACE", "1")

import numpy as np
import ml_dtypes

import concourse.bass as bass
import concourse.bacc as bacc
import concourse.mybir as mybir
import concourse.tile as tile
from concourse.bass_utils import run_bass_kernel_spmd

F32 = mybir.dt.float32
BF16 = mybir.dt.bfloat16
I16 = mybir.dt.int16

B, L, D = 1, 4096, 1024
H, DH = 16, 64
BASE = 10000.0
N_CORES = 8
HPC = H // N_CORES          # heads per core = 2
DH2 = HPC * DH              # packed head dim = 128
SCALE = DH ** -0.5          # 0.125

# Schraudolph-style exp in bf16 bits: bf16(e^(x*SCALE)) ~= bits of
# int16(A*x + B) with A = SCALE * 2^7 / ln2, B = 127*2^7 - 7.41 (minimax).
SCH_A = SCALE * 128.0 / np.log(2.0)
SCH_B = 16256.0 - 7.41


# ---------------------------------------------------------------- host helpers

def _rope_tables(L_, dh):
    inv_freq = 1.0 / (BASE ** (np.arange(0, dh, 2, dtype=np.float32) / dh))
    freqs = np.outer(np.arange(L_, dtype=np.float32), inv_freq)  # [L, 32]
    return np.cos(freqs).astype(np.float32), np.sin(freqs).astype(np.float32)


def _host_consts(L_):
    cos, sin = _rope_tables(L_, DH)          # [L, 32]
    cosT, sinT = cos.T.copy(), sin.T.copy()  # [32, L]
    cos_stack = np.concatenate([cosT, cosT, cosT, cosT], 0)          # [128, L]
    sin_signed = np.concatenate([-sinT, sinT, -sinT, sinT], 0)       # [128, L]

    # 0/1 causal keep-mask within a 128x128 diagonal block:
    # key j visible to query c iff j <= c.
    j = np.arange(128)[:, None]
    c = np.arange(128)[None, :]
    tril01 = (j <= c).astype(np.float32)                             # [128,128]

    ident = np.eye(128, dtype=np.float32)
    # 32-row block swap permutation: out[i] = in[sigma(i)],
    # sigma = [32..63, 0..31, 96..127, 64..95]
    sigma = np.concatenate([np.arange(32, 64), np.arange(0, 32),
                            np.arange(96, 128), np.arange(64, 96)])
    pmat = np.zeros((128, 128), np.float32)
    pmat[sigma, np.arange(128)] = 1.0        # out = pmat.T @ in
    return {
        "cos_st": cos_stack.astype(ml_dtypes.bfloat16),
        "sin_st": sin_signed.astype(ml_dtypes.bfloat16),
        "tril01": tril01.astype(ml_dtypes.bfloat16),
        "ident_b": ident.astype(ml_dtypes.bfloat16),
        "perm_b": pmat.astype(ml_dtypes.bfloat16),
    }


def _chunk_major(wT):
    """[D, 128] -> [128, D] with 128-row chunks laid side by side."""
    ndc = wT.shape[0] // 128
    return np.ascontiguousarray(
        wT.reshape(ndc, 128, 128).transpose(1, 0, 2).reshape(128, ndc * 128))


def _core_weights(core, Wq, Wk, Wv, Wo):
    """Per-core weight slices, bf16, chunk-major; RoPE even/odd permutation
    applied to Wq/Wk rows."""
    perm = np.concatenate([np.arange(0, DH, 2), np.arange(1, DH, 2)])  # [64]
    rows_p, rows = [], []
    for hh in (HPC * core, HPC * core + 1):
        rows_p.append(DH * hh + perm)
        rows.append(DH * hh + np.arange(DH))
    rows_p = np.concatenate(rows_p)
    rows = np.concatenate(rows)
    wq = _chunk_major(Wq[rows_p, :].T).astype(ml_dtypes.bfloat16)  # [128, 1024]
    wk = _chunk_major(Wk[rows_p, :].T).astype(ml_dtypes.bfloat16)
    wv = _chunk_major(Wv[rows, :].T).astype(ml_dtypes.bfloat16)
    woC = np.ascontiguousarray(
        Wo[:, DH2 * core: DH2 * (core + 1)].T).astype(ml_dtypes.bfloat16)
    return wq, wk, wv, woC


def _layout_x(x, L_):
    """x [B,L,D] -> [128, 8*4096] bf16, 512-col subtile-major:
    xr[p, s*4096 + ch*512 + c] = x[s*512+c, ch*128+p]."""
    xT = np.ascontiguousarray(x.reshape(L_, D).T)        # [D, L]
    ns = L_ // 512
    xr = xT.reshape(8, 128, ns, 512).transpose(1, 2, 0, 3)
    return np.ascontiguousarray(xr.reshape(128, ns * 4096)).astype(
        ml_dtypes.bfloat16)


# ---------------------------------------------------------------- device emit

def emit(nc, tc, aps, L_):
    NSB = L_ // 512           # 512-col subtiles (8) == query blocks
    NQB = L_ // 512
    NKB = L_ // 128           # key blocks (32)
    ND = D // 128             # D chunks (8)

    xt = aps["xt"]
    partial = aps["partial"]
    ACT_EXP = mybir.ActivationFunctionType.Exp

    with tc.tile_pool(name="persist", bufs=1) as pp, \
         tc.tile_pool(name="psB", bufs=1, space="PSUM") as psB, \
         tc.tile_pool(name="psS", bufs=1, space="PSUM") as psS, \
         tc.tile_pool(name="sbC", bufs=1) as sbC, \
         tc.tile_pool(name="sbB", bufs=1) as sbB, \
         tc.tile_pool(name="sbA", bufs=1) as sbA:
        wq_sb = pp.tile([128, D], BF16)
        wk_sb = pp.tile([128, D], BF16)
        wv_sb = pp.tile([128, D], BF16)
        wo_sb = pp.tile([128, D], BF16)
        cos_sb = pp.tile([128, L_], BF16)
        sin_sb = pp.tile([128, L_], BF16)
        tril_sb = pp.tile([128, 128], BF16)
        idb_sb = pp.tile([128, 128], BF16)
        perm_sb = pp.tile([128, 128], BF16)
        qT = pp.tile([128, L_], BF16)
        kT = pp.tile([128, L_], BF16)
        v_sb = pp.tile([128, NKB * 130], BF16)
        # PE pstate warmup: the cost model ramps the PE clock over 3us from
        # the first matmul; dummy matmuls during the initial DMA wait start
        # the ramp early so real work runs at mid/full speed sooner.
        wup = pp.tile([128, 256], BF16)
        nc.gpsimd.memset(wup[:], 0.0)
        nc.sync.dma_start(wq_sb[:], aps["wq"][:])
        nc.gpsimd.memset(v_sb[:], 1.0)   # ones columns for the sum trick
        for _ in range(3):
            wps = psS.tile([128, 512], F32, tag="scr", bufs=2)
            nc.tensor.matmul(wps[:, 0:256], wup[:, 0:128], wup[:],
                             start=True, stop=True)

        def phase_a(s):
            """Projections + RoPE + V transpose for L-subtile s (512 cols)."""
            sl = bass.ds(512 * s, 512)
            xt_t = sbA.tile([128, 4096], BF16, tag="xt", bufs=2)
            if s == 0:
                for ch in range(ND):
                    nc.sync.dma_start(xt_t[:, bass.ts(ch, 512)],
                                      xt[:, bass.ds(ch * 512, 512)])
                nc.sync.dma_start(wk_sb[:], aps["wk"][:])
                nc.sync.dma_start(wv_sb[:], aps["wv"][:])
                # only the first 512-col slice of cos/sin is needed for s=0;
                # the rest streams in behind xt(1) to unblock it
                nc.sync.dma_start(cos_sb[:, 0:512], aps["cos_st"][:, 0:512])
                nc.sync.dma_start(sin_sb[:, 0:512], aps["sin_st"][:, 0:512])
                nc.sync.dma_start(perm_sb[:], aps["perm_b"][:])
                nc.sync.dma_start(idb_sb[:], aps["ident_b"][:])
                nc.sync.dma_start(tril_sb[:], aps["tril01"][:])
            else:
                nc.sync.dma_start(xt_t[:], xt[:, bass.ts(s, 4096)])
                if s == 1:
                    nc.sync.dma_start(cos_sb[:, 512:L_],
                                      aps["cos_st"][:, 512:L_])
                    nc.sync.dma_start(sin_sb[:, 512:L_],
                                      aps["sin_st"][:, 512:L_])
                    nc.sync.dma_start(wo_sb[:], aps["wo"][:])
            raws = {}
            # q/k projections first so their RoPE (DVE) overlaps the
            # V projection + transposes (PE) and attention(s) starts clean.
            for name, wsb in (("q", wq_sb), ("k", wk_sb)):
                ps = psS.tile([128, 512], F32, tag="scr", bufs=2)
                for ch in range(ND):
                    nc.tensor.matmul(ps[:], wsb[:, bass.ts(ch, 128)],
                                     xt_t[:, bass.ts(ch, 512)],
                                     start=ch == 0, stop=ch == ND - 1)
                raw = sbA.tile([128, 512], BF16, tag=f"raw{name}", bufs=2)
                if s <= 3:
                    nc.scalar.copy(raw[:], ps[:])   # ACT is starved early
                else:
                    nc.vector.tensor_copy(raw[:], ps[:])
                raws[name] = raw
            # RoPE: rot = raw*cos + perm(raw)*sin_signed
            for name, dst in (("q", qT), ("k", kT)):
                raw = raws[name]
                aux = psS.tile([128, 512], F32, tag="scr", bufs=2)
                nc.tensor.matmul(aux[:], perm_sb[:], raw[:],
                                 start=True, stop=True)
                swp = sbA.tile([128, 512], BF16, tag="swp", bufs=2)
                nc.vector.tensor_mul(swp[:], aux[:], sin_sb[:, sl])
                nc.vector.tensor_mul(dst[:, sl], raw[:], cos_sb[:, sl])
                nc.vector.tensor_add(dst[:, sl], dst[:, sl], swp[:])
            psv = psS.tile([128, 512], F32, tag="scr", bufs=2)
            for ch in range(ND):
                nc.tensor.matmul(psv[:], wv_sb[:, bass.ts(ch, 128)],
                                 xt_t[:, bass.ts(ch, 512)],
                                 start=ch == 0, stop=ch == ND - 1)
            vt = sbA.tile([128, 512], BF16, tag="rawv", bufs=2)
            nc.vector.tensor_copy(vt[:], psv[:])
            # V transpose into [key, dh] layout with ones columns:
            # v_sb[:, 130*kb + {0..63, 65..128}], kb = 4*s + j
            auxv_t = psS.tile([128, 512], F32, tag="scr", bufs=2)
            auxv = auxv_t[:].bitcast(BF16)[:, 0:512]
            for j in range(4):
                nc.tensor.transpose(auxv[:, bass.ts(j, 128)],
                                    vt[:, bass.ts(j, 128)], idb_sb[:])
            src = auxv.rearrange("p (j h c) -> p j h c", j=4, h=2)
            vdst = v_sb[:, bass.ds(130 * 4 * s, 130 * 4)].rearrange(
                "p (j h c) -> p j h c", j=4, c=65)[:, :, :, 0:64]
            nc.vector.tensor_copy(vdst, src)

        state = {}  # qb -> (O_sb, invs) for deferred norm/o_proj

        def norm_oproj(qb, use_act=False):
            O_t, invs = state.pop(qb)
            # use_act: ACT is free of exp work here; alternate ACT/DVE so
            # neither engine serializes the whole copy chain
            def cpy(dst, src, n=[0]):
                n[0] ^= 1
                if use_act and n[0]:
                    nc.scalar.copy(dst, src)
                else:
                    nc.vector.tensor_copy(dst, src)
            for qs in range(4):
                lc = 4 * qb + qs
                for h in range(2):
                    i = 2 * qs + h
                    nc.vector.tensor_scalar_mul(
                        O_t[:, bass.ds(64 * i, 64)],
                        O_t[:, bass.ds(64 * i, 64)],
                        invs[:, i:i + 1])
                trp_t = psS.tile([128, 512], F32, tag="scr", bufs=2)
                trpb = trp_t[:].bitcast(BF16)[:, 0:128]
                nc.tensor.transpose(trpb, O_t[:, bass.ts(qs, 128)],
                                    idb_sb[:])
                ot_t = sbC.tile([128, 128], BF16, tag="ot", bufs=2)
                nc.vector.tensor_copy(ot_t[:], trpb)
                ob = sbC.tile([128, 1024], BF16, tag="ob", bufs=4)
                for n in range(2):
                    op = psS.tile([128, 512], F32, tag="scr", bufs=2)
                    nc.tensor.matmul(op[:], ot_t[:],
                                     wo_sb[:, bass.ts(n, 512)],
                                     start=True, stop=True)
                    cpy(ob[:, bass.ts(n, 512)], op[:])
                # tail DMAs ride the idle HWDGE path; mid-stream ones stay on
                # SWDGE to keep HWDGE free for input streaming
                if use_act:
                    nc.sync.dma_start(partial[bass.ts(lc, 128), :], ob[:])
                else:
                    nc.gpsimd.dma_start(partial[bass.ts(lc, 128), :], ob[:])

        # PSUM accumulate-group state is per bank: region 7 would cross
        # the 2048B bank boundary at col 455, so it lives at col 512.
        PVC = [65 * i for i in range(7)] + [512]

        def attention(qb):
            qsl0 = 512 * qb
            pvacc = psB.tile([128, 577], F32, tag="pv", bufs=1)
            # no zeroing matmuls: the first PV into each bank (kb=0, i=0 for
            # bank 0 / i=7 for bank 1) carries start=True, clearing the
            # bank's has_written; later regions then write in overwrite mode
            nkb = 4 * qb + 4
            for kb in range(nkb):
                r = kb - 4 * qb
                c0 = 128 * r if r > 0 else 0
                W = 512 - c0
                ksl = bass.ts(kb, 128)
                qsl = bass.ds(qsl0 + c0, W)
                s01 = psB.tile([128, 1024], F32, tag="sc", bufs=2)
                nc.tensor.matmul(s01[:, c0:512], kT[0:64, ksl],
                                 qT[0:64, qsl], start=True, stop=True)
                nc.tensor.matmul(s01[:, 512 + c0:1024], kT[64:128, ksl],
                                 qT[64:128, qsl], start=True, stop=True)
                p01 = sbB.tile([128, 1024], BF16, tag="p01", bufs=8)
                if qb >= 6 and r < 0 and kb % 4 == 3:
                    # offload exp to DVE via bf16 Schraudolph bit trick
                    nc.vector.tensor_scalar(
                        p01[:].bitcast(I16), s01[:], SCH_A, SCH_B,
                        mybir.AluOpType.mult, mybir.AluOpType.add)
                else:
                    sin_ = s01[:].rearrange(
                        "p (h c) -> p h c", h=2)[:, :, c0:512]
                    pout = p01[:].rearrange(
                        "p (h c) -> p h c", h=2)[:, :, c0:512]
                    nc.scalar.activation(pout, sin_, ACT_EXP, scale=SCALE)
                if r >= 0:
                    for h in range(2):
                        msl = bass.ds(512 * h + c0, 128)
                        nc.vector.tensor_mul(p01[:, msl], p01[:, msl],
                                             tril_sb[:])
                for qs in range(max(0, r), 4):
                    for h in range(2):
                        i = 2 * qs + h
                        nc.tensor.matmul(
                            pvacc[:, bass.ds(PVC[i], 65)],
                            p01[:, bass.ds(512 * h + 128 * qs, 128)],
                            v_sb[:, bass.ds(130 * kb + 65 * h, 65)],
                            start=(kb == 0 and i in (0, 7)),
                            stop=kb == 4 * qb + qs,
                            skip_group_check=True)
            # free pvacc quickly: reciprocal of sums + copy out
            invs = sbB.tile([128, 8], F32, tag="invs", bufs=3)
            sums7 = pvacc[:, 0:455].rearrange(
                "p (i c) -> p i c", c=65)[:, :, 64]
            nc.vector.reciprocal(invs[:, 0:7], sums7)
            nc.vector.reciprocal(invs[:, 7:8], pvacc[:, 576:577])
            O_t = sbB.tile([128, 512], BF16, tag="osb", bufs=3)
            psrc7 = pvacc[:, 0:455].rearrange(
                "p (i c) -> p i c", c=65)[:, :, 0:64]
            nc.vector.tensor_copy(
                O_t[:, 0:448].rearrange("p (i c) -> p i c", c=64), psrc7)
            nc.vector.tensor_copy(O_t[:, 448:512], pvacc[:, 512:576])
            state[qb] = (O_t, invs)

        # Interleave: attention row qb needs keys 0..512*(qb+1) = subtiles
        # 0..qb, so row s can run right after phase_a(s).
        for s in range(NSB):
            phase_a(s)
            attention(s)
            if s >= 1:
                norm_oproj(s - 1)
        norm_oproj(NQB - 1, use_act=True)


def build(L_=L, debug=False):
    nc = bacc.Bacc("TRN2", target_bir_lowering=False, debug=debug,
                   enable_asserts=False)
    aps = {}
    NSB = L_ // 512
    aps["xt"] = nc.dram_tensor("xt", [128, NSB * 4096], BF16,
                               kind="ExternalInput").ap()
    for w in ("wq", "wk", "wv", "wo"):
        aps[w] = nc.dram_tensor(w, [128, D], BF16, kind="ExternalInput").ap()
    aps["cos_st"] = nc.dram_tensor("cos_st", [128, L_], BF16,
                                   kind="ExternalInput").ap()
    aps["sin_st"] = nc.dram_tensor("sin_st", [128, L_], BF16,
                                   kind="ExternalInput").ap()
    aps["tril01"] = nc.dram_tensor("tril01", [128, 128], BF16,
                                   kind="ExternalInput").ap()
    aps["ident_b"] = nc.dram_tensor("ident_b", [128, 128], BF16,
                                    kind="ExternalInput").ap()
    aps["perm_b"] = nc.dram_tensor("perm_b", [128, 128], BF16,
                                   kind="ExternalInput").ap()
    aps["partial"] = nc.dram_tensor("partial", [L_, D], BF16,
                                    kind="ExternalOutput").ap()

    with tile.TileContext(nc) as tc:
        emit(nc, tc, aps, L_)
    nc.compile()
    return nc, aps


def make_in_maps(x, Wq, Wk, Wv, Wo, L_=L):
    xr = _layout_x(x, L_)
    consts = _host_consts(L_)
    in_maps = []
    for c in range(N_CORES):
        wq, wk, wv, woC = _core_weights(c, Wq, Wk, Wv, Wo)
        m = {"xt": xr, "wq": wq, "wk": wk, "wv": wv, "wo": woC}
        m.update(consts)
        in_maps.append(m)
    return in_maps


_CACHE = {}


def _run(inputs, trace=False, **kw):
    if trace:
        os.environ.pop("BASS_NEVER_TRACE", None)
    x = np.asarray(inputs["x"], np.float32)
    Wq = np.asarray(inputs["Wq"], np.float32)
    Wk = np.asarray(inputs["Wk"], np.float32)
    Wv = np.asarray(inputs["Wv"], np.float32)
    Wo = np.asarray(inputs["Wo"], np.float32)
    if "nc" not in _CACHE:
        _CACHE["nc"] = build()[0]
    nc = _CACHE["nc"]
    in_maps = make_in_maps(x, Wq, Wk, Wv, Wo)
    res = run_bass_kernel_spmd(nc, in_maps, core_ids=list(range(N_CORES)),
                               trace=trace, **kw)
    acc = np.zeros((L, D), np.float64)
    for r in res.results:
        acc += r["partial"].astype(np.float64)
    out = acc.astype(np.float32).reshape(B, L, D)
    return out, res


def kernel(**inputs):
    out, _ = _run(inputs)
    return out


# revision 83
# speedup vs baseline: 1.0040x; 1.0001x over previous
"""Trainium2 Bass kernel for nn_BloqueAttn: causal RoPE attention, 16 heads,
head-sharded (tensor-parallel) across 8 NeuronCores, o_proj row-sharded with
host-side all-reduce of the partials.

v2: bf16 datapath, query-on-partition PV (65-wide moving operand), PE
perm-matmul RoPE swap, mask-by-multiply on DVE, per-partition softmax
normalization, batched DMAs with host-side pre-layout.

Self-contained: hardcodes shapes B=1, L=4096, D=1024, H=16, DH=64, 8 cores.
"""
import os

os.environ.setdefault("BASS_NEVER_TRACE", "1")

import numpy as np
import ml_dtypes

import concourse.bass as bass
import concourse.bacc as bacc
import concourse.mybir as mybir
import concourse.tile as tile
from concourse.bass_utils import run_bass_kernel_spmd

F32 = mybir.dt.float32
BF16 = mybir.dt.bfloat16
I16 = mybir.dt.int16

B, L, D = 1, 4096, 1024
H, DH = 16, 64
BASE = 10000.0
N_CORES = 8
HPC = H // N_CORES          # heads per core = 2
DH2 = HPC * DH              # packed head dim = 128
SCALE = DH ** -0.5          # 0.125

# Schraudolph-style exp in bf16 bits: bf16(e^(x*SCALE)) ~= bits of
# int16(A*x + B) with A = SCALE * 2^7 / ln2, B = 127*2^7 - 7.41 (minimax).
SCH_A = SCALE * 128.0 / np.log(2.0)
SCH_B = 16256.0 - 7.41


# ---------------------------------------------------------------- host helpers

def _rope_tables(L_, dh):
    inv_freq = 1.0 / (BASE ** (np.arange(0, dh, 2, dtype=np.float32) / dh))
    freqs = np.outer(np.arange(L_, dtype=np.float32), inv_freq)  # [L, 32]
    return np.cos(freqs).astype(np.float32), np.sin(freqs).astype(np.float32)


def _host_consts(L_):
    cos, sin = _rope_tables(L_, DH)          # [L, 32]
    cosT, sinT = cos.T.copy(), sin.T.copy()  # [32, L]
    cos_stack = np.concatenate([cosT, cosT, cosT, cosT], 0)          # [128, L]
    sin_signed = np.concatenate([-sinT, sinT, -sinT, sinT], 0)       # [128, L]

    # 0/1 causal keep-mask within a 128x128 diagonal block:
    # key j visible to query c iff j <= c.
    j = np.arange(128)[:, None]
    c = np.arange(128)[None, :]
    tril01 = (j <= c).astype(np.float32)                             # [128,128]

    ident = np.eye(128, dtype=np.float32)
    # 32-row block swap permutation: out[i] = in[sigma(i)],
    # sigma = [32..63, 0..31, 96..127, 64..95]
    sigma = np.concatenate([np.arange(32, 64), np.arange(0, 32),
                            np.arange(96, 128), np.arange(64, 96)])
    pmat = np.zeros((128, 128), np.float32)
    pmat[sigma, np.arange(128)] = 1.0        # out = pmat.T @ in
    return {
        "cos_st": cos_stack.astype(ml_dtypes.bfloat16),
        "sin_st": sin_signed.astype(ml_dtypes.bfloat16),
        "tril01": tril01.astype(ml_dtypes.bfloat16),
        "ident_b": ident.astype(ml_dtypes.bfloat16),
        "perm_b": pmat.astype(ml_dtypes.bfloat16),
    }


def _chunk_major(wT):
    """[D, 128] -> [128, D] with 128-row chunks laid side by side."""
    ndc = wT.shape[0] // 128
    return np.ascontiguousarray(
        wT.reshape(ndc, 128, 128).transpose(1, 0, 2).reshape(128, ndc * 128))


def _core_weights(core, Wq, Wk, Wv, Wo):
    """Per-core weight slices, bf16, chunk-major; RoPE even/odd permutation
    applied to Wq/Wk rows."""
    perm = np.concatenate([np.arange(0, DH, 2), np.arange(1, DH, 2)])  # [64]
    rows_p, rows = [], []
    for hh in (HPC * core, HPC * core + 1):
        rows_p.append(DH * hh + perm)
        rows.append(DH * hh + np.arange(DH))
    rows_p = np.concatenate(rows_p)
    rows = np.concatenate(rows)
    wq = _chunk_major(Wq[rows_p, :].T).astype(ml_dtypes.bfloat16)  # [128, 1024]
    wk = _chunk_major(Wk[rows_p, :].T).astype(ml_dtypes.bfloat16)
    wv = _chunk_major(Wv[rows, :].T).astype(ml_dtypes.bfloat16)
    woC = np.ascontiguousarray(
        Wo[:, DH2 * core: DH2 * (core + 1)].T).astype(ml_dtypes.bfloat16)
    return wq, wk, wv, woC


def _layout_x(x, L_):
    """x [B,L,D] -> [128, 8*4096] bf16, 512-col subtile-major:
    xr[p, s*4096 + ch*512 + c] = x[s*512+c, ch*128+p]."""
    xT = np.ascontiguousarray(x.reshape(L_, D).T)        # [D, L]
    ns = L_ // 512
    xr = xT.reshape(8, 128, ns, 512).transpose(1, 2, 0, 3)
    return np.ascontiguousarray(xr.reshape(128, ns * 4096)).astype(
        ml_dtypes.bfloat16)


# ---------------------------------------------------------------- device emit

def emit(nc, tc, aps, L_):
    NSB = L_ // 512           # 512-col subtiles (8) == query blocks
    NQB = L_ // 512
    NKB = L_ // 128           # key blocks (32)
    ND = D // 128             # D chunks (8)

    xt = aps["xt"]
    partial = aps["partial"]
    ACT_EXP = mybir.ActivationFunctionType.Exp

    with tc.tile_pool(name="persist", bufs=1) as pp, \
         tc.tile_pool(name="psB", bufs=1, space="PSUM") as psB, \
         tc.tile_pool(name="psS", bufs=1, space="PSUM") as psS, \
         tc.tile_pool(name="sbC", bufs=1) as sbC, \
         tc.tile_pool(name="sbB", bufs=1) as sbB, \
         tc.tile_pool(name="sbA", bufs=1) as sbA:
        wq_sb = pp.tile([128, D], BF16)
        wk_sb = pp.tile([128, D], BF16)
        wv_sb = pp.tile([128, D], BF16)
        wo_sb = pp.tile([128, D], BF16)
        cos_sb = pp.tile([128, L_], BF16)
        sin_sb = pp.tile([128, L_], BF16)
        tril_sb = pp.tile([128, 128], BF16)
        idb_sb = pp.tile([128, 128], BF16)
        perm_sb = pp.tile([128, 128], BF16)
        qT = pp.tile([128, L_], BF16)
        kT = pp.tile([128, L_], BF16)
        v_sb = pp.tile([128, NKB * 130], BF16)
        # PE pstate warmup: the cost model ramps the PE clock over 3us from
        # the first matmul; dummy matmuls during the initial DMA wait start
        # the ramp early so real work runs at mid/full speed sooner.
        wup = pp.tile([128, 256], BF16)
        nc.gpsimd.memset(wup[:], 0.0)
        nc.sync.dma_start(wq_sb[:], aps["wq"][:])
        nc.gpsimd.memset(v_sb[:], 1.0)   # ones columns for the sum trick
        for _ in range(3):
            wps = psS.tile([128, 512], F32, tag="scr", bufs=2)
            nc.tensor.matmul(wps[:, 0:256], wup[:, 0:128], wup[:],
                             start=True, stop=True)

        def phase_a(s):
            """Projections + RoPE + V transpose for L-subtile s (512 cols)."""
            sl = bass.ds(512 * s, 512)
            xt_t = sbA.tile([128, 4096], BF16, tag="xt", bufs=2)
            if s == 0:
                for ch in range(ND):
                    nc.sync.dma_start(xt_t[:, bass.ts(ch, 512)],
                                      xt[:, bass.ds(ch * 512, 512)])
                nc.sync.dma_start(wk_sb[:], aps["wk"][:])
                nc.sync.dma_start(wv_sb[:], aps["wv"][:])
                # only the first 512-col slice of cos/sin is needed for s=0;
                # the rest streams in behind xt(1) to unblock it
                nc.sync.dma_start(cos_sb[:, 0:512], aps["cos_st"][:, 0:512])
                nc.sync.dma_start(sin_sb[:, 0:512], aps["sin_st"][:, 0:512])
                nc.sync.dma_start(perm_sb[:], aps["perm_b"][:])
                nc.sync.dma_start(idb_sb[:], aps["ident_b"][:])
                nc.sync.dma_start(tril_sb[:], aps["tril01"][:])
            else:
                nc.sync.dma_start(xt_t[:], xt[:, bass.ts(s, 4096)])
                if s == 1:
                    nc.sync.dma_start(cos_sb[:, 512:L_],
                                      aps["cos_st"][:, 512:L_])
                    nc.sync.dma_start(sin_sb[:, 512:L_],
                                      aps["sin_st"][:, 512:L_])
                    nc.sync.dma_start(wo_sb[:], aps["wo"][:])
            raws = {}
            # q/k projections first so their RoPE (DVE) overlaps the
            # V projection + transposes (PE) and attention(s) starts clean.
            for name, wsb in (("q", wq_sb), ("k", wk_sb)):
                ps = psS.tile([128, 512], F32, tag="scr", bufs=2)
                for ch in range(ND):
                    nc.tensor.matmul(ps[:], wsb[:, bass.ts(ch, 128)],
                                     xt_t[:, bass.ts(ch, 512)],
                                     start=ch == 0, stop=ch == ND - 1)
                raw = sbA.tile([128, 512], BF16, tag=f"raw{name}", bufs=2)
                if s <= 3:
                    nc.scalar.copy(raw[:], ps[:])   # ACT is starved early
                else:
                    nc.vector.tensor_copy(raw[:], ps[:])
                raws[name] = raw
            # RoPE: rot = raw*cos + perm(raw)*sin_signed
            for name, dst in (("q", qT), ("k", kT)):
                raw = raws[name]
                aux = psS.tile([128, 512], F32, tag="scr", bufs=2)
                nc.tensor.matmul(aux[:], perm_sb[:], raw[:],
                                 start=True, stop=True)
                swp = sbA.tile([128, 512], BF16, tag="swp", bufs=2)
                nc.vector.tensor_mul(swp[:], aux[:], sin_sb[:, sl])
                nc.vector.tensor_mul(dst[:, sl], raw[:], cos_sb[:, sl])
                nc.vector.tensor_add(dst[:, sl], dst[:, sl], swp[:])
            psv = psS.tile([128, 512], F32, tag="scr", bufs=2)
            for ch in range(ND):
                nc.tensor.matmul(psv[:], wv_sb[:, bass.ts(ch, 128)],
                                 xt_t[:, bass.ts(ch, 512)],
                                 start=ch == 0, stop=ch == ND - 1)
            vt = sbA.tile([128, 512], BF16, tag="rawv", bufs=2)
            nc.vector.tensor_copy(vt[:], psv[:])
            # V transpose into [key, dh] layout with ones columns:
            # v_sb[:, 130*kb + {0..63, 65..128}], kb = 4*s + j
            auxv_t = psS.tile([128, 512], F32, tag="scr", bufs=2)
            auxv = auxv_t[:].bitcast(BF16)[:, 0:512]
            for j in range(4):
                nc.tensor.transpose(auxv[:, bass.ts(j, 128)],
                                    vt[:, bass.ts(j, 128)], idb_sb[:])
            src = auxv.rearrange("p (j h c) -> p j h c", j=4, h=2)
            vdst = v_sb[:, bass.ds(130 * 4 * s, 130 * 4)].rearrange(
                "p (j h c) -> p j h c", j=4, c=65)[:, :, :, 0:64]
            nc.vector.tensor_copy(vdst, src)

        state = {}  # qb -> (O_sb, invs) for deferred norm/o_proj

        def norm_oproj(qb, use_act=False):
            O_t, invs = state.pop(qb)
            # use_act: ACT is free of exp work here; alternate ACT/DVE so
            # neither engine serializes the whole copy chain
            def cpy(dst, src, n=[0]):
                n[0] ^= 1
                if use_act and n[0]:
                    nc.scalar.copy(dst, src)
                else:
                    nc.vector.tensor_copy(dst, src)
            for qs in range(4):
                lc = 4 * qb + qs
                for h in range(2):
                    i = 2 * qs + h
                    nc.vector.tensor_scalar_mul(
                        O_t[:, bass.ds(64 * i, 64)],
                        O_t[:, bass.ds(64 * i, 64)],
                        invs[:, i:i + 1])
                trp_t = psS.tile([128, 512], F32, tag="scr", bufs=2)
                trpb = trp_t[:].bitcast(BF16)[:, 0:128]
                nc.tensor.transpose(trpb, O_t[:, bass.ts(qs, 128)],
                                    idb_sb[:])
                ot_t = sbC.tile([128, 128], BF16, tag="ot", bufs=3)
                nc.vector.tensor_copy(ot_t[:], trpb)
                ob = sbC.tile([128, 1024], BF16, tag="ob", bufs=5)
                for n in range(2):
                    op = psS.tile([128, 512], F32, tag="scr", bufs=2)
                    nc.tensor.matmul(op[:], ot_t[:],
                                     wo_sb[:, bass.ts(n, 512)],
                                     start=True, stop=True)
                    cpy(ob[:, bass.ts(n, 512)], op[:])
                # tail DMAs ride the idle HWDGE path; mid-stream ones stay on
                # SWDGE to keep HWDGE free for input streaming
                if use_act:
                    nc.sync.dma_start(partial[bass.ts(lc, 128), :], ob[:])
                else:
                    nc.gpsimd.dma_start(partial[bass.ts(lc, 128), :], ob[:])

        # PSUM accumulate-group state is per bank: region 7 would cross
        # the 2048B bank boundary at col 455, so it lives at col 512.
        PVC = [65 * i for i in range(7)] + [512]

        def attention(qb):
            qsl0 = 512 * qb
            pvacc = psB.tile([128, 577], F32, tag="pv", bufs=1)
            # no zeroing matmuls: the first PV into each bank (kb=0, i=0 for
            # bank 0 / i=7 for bank 1) carries start=True, clearing the
            # bank's has_written; later regions then write in overwrite mode
            nkb = 4 * qb + 4
            for kb in range(nkb):
                r = kb - 4 * qb
                c0 = 128 * r if r > 0 else 0
                W = 512 - c0
                ksl = bass.ts(kb, 128)
                qsl = bass.ds(qsl0 + c0, W)
                s01 = psB.tile([128, 1024], F32, tag="sc", bufs=2)
                nc.tensor.matmul(s01[:, c0:512], kT[0:64, ksl],
                                 qT[0:64, qsl], start=True, stop=True)
                nc.tensor.matmul(s01[:, 512 + c0:1024], kT[64:128, ksl],
                                 qT[64:128, qsl], start=True, stop=True)
                p01 = sbB.tile([128, 1024], BF16, tag="p01", bufs=10)
                if qb >= 6 and r < 0 and kb % 4 == 3:
                    # offload exp to DVE via bf16 Schraudolph bit trick
                    nc.vector.tensor_scalar(
                        p01[:].bitcast(I16), s01[:], SCH_A, SCH_B,
                        mybir.AluOpType.mult, mybir.AluOpType.add)
                else:
                    sin_ = s01[:].rearrange(
                        "p (h c) -> p h c", h=2)[:, :, c0:512]
                    pout = p01[:].rearrange(
                        "p (h c) -> p h c", h=2)[:, :, c0:512]
                    nc.scalar.activation(pout, sin_, ACT_EXP, scale=SCALE)
                if r >= 0:
                    for h in range(2):
                        msl = bass.ds(512 * h + c0, 128)
                        nc.vector.tensor_mul(p01[:, msl], p01[:, msl],
                                             tril_sb[:])
                for qs in range(max(0, r), 4):
                    for h in range(2):
                        i = 2 * qs + h
                        nc.tensor.matmul(
                            pvacc[:, bass.ds(PVC[i], 65)],
                            p01[:, bass.ds(512 * h + 128 * qs, 128)],
                            v_sb[:, bass.ds(130 * kb + 65 * h, 65)],
                            start=(kb == 0 and i in (0, 7)),
                            stop=kb == 4 * qb + qs,
                            skip_group_check=True)
            # free pvacc quickly: reciprocal of sums + copy out
            invs = sbB.tile([128, 8], F32, tag="invs", bufs=3)
            sums7 = pvacc[:, 0:455].rearrange(
                "p (i c) -> p i c", c=65)[:, :, 64]
            nc.vector.reciprocal(invs[:, 0:7], sums7)
            nc.vector.reciprocal(invs[:, 7:8], pvacc[:, 576:577])
            O_t = sbB.tile([128, 512], BF16, tag="osb", bufs=3)
            psrc7 = pvacc[:, 0:455].rearrange(
                "p (i c) -> p i c", c=65)[:, :, 0:64]
            nc.vector.tensor_copy(
                O_t[:, 0:448].rearrange("p (i c) -> p i c", c=64), psrc7)
            nc.vector.tensor_copy(O_t[:, 448:512], pvacc[:, 512:576])
            state[qb] = (O_t, invs)

        # Interleave: attention row qb needs keys 0..512*(qb+1) = subtiles
        # 0..qb, so row s can run right after phase_a(s).
        for s in range(NSB):
            phase_a(s)
            attention(s)
            if s >= 1:
                norm_oproj(s - 1)
        norm_oproj(NQB - 1, use_act=True)


def build(L_=L, debug=False):
    nc = bacc.Bacc("TRN2", target_bir_lowering=False, debug=debug,
                   enable_asserts=False)
    aps = {}
    NSB = L_ // 512
    aps["xt"] = nc.dram_tensor("xt", [128, NSB * 4096], BF16,
                               kind="ExternalInput").ap()
    for w in ("wq", "wk", "wv", "wo"):
        aps[w] = nc.dram_tensor(w, [128, D], BF16, kind="ExternalInput").ap()
    aps["cos_st"] = nc.dram_tensor("cos_st", [128, L_], BF16,
                                   kind="ExternalInput").ap()
    aps["sin_st"] = nc.dram_tensor("sin_st", [128, L_], BF16,
                                   kind="ExternalInput").ap()
    aps["tril01"] = nc.dram_tensor("tril01", [128, 128], BF16,
                                   kind="ExternalInput").ap()
    aps["ident_b"] = nc.dram_tensor("ident_b", [128, 128], BF16,
                                    kind="ExternalInput").ap()
    aps["perm_b"] = nc.dram_tensor("perm_b", [128, 128], BF16,
                                   kind="ExternalInput").ap()
    aps["partial"] = nc.dram_tensor("partial", [L_, D], BF16,
                                    kind="ExternalOutput").ap()

    with tile.TileContext(nc) as tc:
        emit(nc, tc, aps, L_)
    nc.compile()
    return nc, aps


def make_in_maps(x, Wq, Wk, Wv, Wo, L_=L):
    xr = _layout_x(x, L_)
    consts = _host_consts(L_)
    in_maps = []
    for c in range(N_CORES):
        wq, wk, wv, woC = _core_weights(c, Wq, Wk, Wv, Wo)
        m = {"xt": xr, "wq": wq, "wk": wk, "wv": wv, "wo": woC}
        m.update(consts)
        in_maps.append(m)
    return in_maps


_CACHE = {}


def _run(inputs, trace=False, **kw):
    if trace:
        os.environ.pop("BASS_NEVER_TRACE", None)
    x = np.asarray(inputs["x"], np.float32)
    Wq = np.asarray(inputs["Wq"], np.float32)
    Wk = np.asarray(inputs["Wk"], np.float32)
    Wv = np.asarray(inputs["Wv"], np.float32)
    Wo = np.asarray(inputs["Wo"], np.float32)
    if "nc" not in _CACHE:
        _CACHE["nc"] = build()[0]
    nc = _CACHE["nc"]
    in_maps = make_in_maps(x, Wq, Wk, Wv, Wo)
    res = run_bass_kernel_spmd(nc, in_maps, core_ids=list(range(N_CORES)),
                               trace=trace, **kw)
    acc = np.zeros((L, D), np.float64)
    for r in res.results:
        acc += r["partial"].astype(np.float64)
    out = acc.astype(np.float32).reshape(B, L, D)
    return out, res


def kernel(**inputs):
    out, _ = _run(inputs)
    return out
